# revision 1
# baseline (speedup 1.0000x reference)
"""Trainium2 Bass kernel for the YAT MixerBlock (nn_MixerBlock_12524124635797).

Strategy: pure data-parallel over batch (64 -> 8 per core). Each core runs
the full mixer block for its 8 batch elements.

Per-core dataflow (all GEMMs fp16 inputs, fp32 PSUM accumulation):
  Token stage (per batch b, x_b is (196p, 768c)):
    dot1 (384t-part, 768c-free) = twT.T @ x_b            [PE]
    den  = wn_t[t] + xn[c] - 2*dot1 + eps                [DVE affine_then_add]
    rec  = 1/den                                         [DVE reciprocal_approx_fast]
    sq   = (dot1 + tb[t])^2                              [ACT Square, bias slot]
    h1   = sq * rec  (fp16)                              [GPSIMD mult; scale_t folded into w2]
    x2T (768c-part, 196p-free) = h1.T@w2sT + x_b.T@I196 + ones.T@b2row   [PE, shortcut+bias
                                                          folded in as extra K rows]
  Channel stage (rows = (b,p) flattened, 1568 per core):
    xn2b (128, rows) = ones.T @ (x2T*x2T)                [PE broadcast of row norms]
    for row-block rb, for m-chunk mc (24 chunks of 3072):
      dot2 (128m-part, rows-free) = cwT.T @ x2T          [PE]
      den2/rec2/sq2/h2 as above (wn_c, cb per-partition) [DVE/ACT/GPSIMD]
      out_psum(rows-part, 768c) += h2.T @ w4sT[mc]       [PE]
    out_psum += x2T.T @ I768 + ones.T @ b4row            [PE, shortcut+bias]
    out (rows, 768) fp32 -> DRAM                         [ACT copy + DMA]
"""

import numpy as np

import concourse.bass as bass
import concourse.bacc as bacc
import concourse.mybir as mybir
from concourse import bass_utils
from concourse import tile

F16 = mybir.dt.float16
F32 = mybir.dt.float32
AF = mybir.ActivationFunctionType

EPS = 0.1
B, P, C, T, M3 = 64, 196, 768, 384, 3072
NCORES = 8
BL = B // NCORES          # 8 batches per core
ROWS = BL * P             # 1568 rows per core
ROWSP = 1664              # ROWS padded to a multiple of 128
RB = 256                  # row-block size for the channel stage (2 psum chunks)


def _ceil_div(a, b):
    return (a + b - 1) // b


def _n_slices(n, step=512):
    """Split [0, n) into matmul-legal free-dim slices (<=512, bank-aligned)."""
    out = []
    o = 0
    while o < n:
        out.append((o, min(step, n - o)))
        o += step
    return out


def build_program():
    nc = bacc.Bacc(
        "TRN2",
        target_bir_lowering=False,
        debug=False,
        enable_asserts=False,
        num_devices=NCORES,
    )

    # ---- DRAM I/O ----
    d = {}
    d["xa"] = nc.dram_tensor("xa", [BL, 128, C], F16, kind="ExternalInput").ap()
    d["xb"] = nc.dram_tensor("xb", [BL, 128, C], F16, kind="ExternalInput").ap()
    d["twT"] = nc.dram_tensor("twT", [128, 2, T], F16, kind="ExternalInput").ap()
    d["w2sT"] = nc.dram_tensor("w2sT", [128, 3, P], F16, kind="ExternalInput").ap()
    d["i196"] = nc.dram_tensor("i196", [128, 2, P], F16, kind="ExternalInput").ap()
    d["b2r"] = nc.dram_tensor("b2r", [1, P], F16, kind="ExternalInput").ap()
    d["cwT"] = nc.dram_tensor("cwT", [128, 6, M3], F16, kind="ExternalInput").ap()
    d["w4sT"] = nc.dram_tensor("w4sT", [128, 24, C], F16, kind="ExternalInput").ap()
    d["b4r"] = nc.dram_tensor("b4r", [1, C], F16, kind="ExternalInput").ap()
    d["wnt"] = nc.dram_tensor("wnt", [128, 3], F32, kind="ExternalInput").ap()
    d["tbc"] = nc.dram_tensor("tbc", [128, 3], F32, kind="ExternalInput").ap()
    d["wnc"] = nc.dram_tensor("wnc", [128, 24], F32, kind="ExternalInput").ap()
    d["cbc"] = nc.dram_tensor("cbc", [128, 24], F32, kind="ExternalInput").ap()
    out_dram = nc.dram_tensor("out", [ROWS, C], F16, kind="ExternalOutput").ap()

    with tile.TileContext(nc) as tc:
        with tc.tile_pool(name="consts", bufs=1) as cp:
            # Resident constants / persistent activations.
            twT = cp.tile([128, 2, T], F16)
            w2sT = cp.tile([128, 3, P], F16)
            i196 = cp.tile([128, 2, P], F16)
            b2r = cp.tile([128, P], F16)
            cwT = cp.tile([128, 6, M3], F16)
            w4sT = cp.tile([128, 24, C], F16)
            b4r = cp.tile([128, C], F16)
            wnt = cp.tile([128, 3], F32)
            tbc = cp.tile([128, 3], F32)
            wnc = cp.tile([128, 24], F32)
            cbc = cp.tile([128, 24], F32)
            ones = cp.tile([128, 128], F16)
            # Free dim padded to a multiple of 128 so the tail row-block's
            # 128-col DMA transpose reads stay in bounds (garbage cols unused).
            x2T = cp.tile([128, 6, ROWSP], F16)
            xn2b = cp.tile([128, ROWS], F32)

            # x input first (token stage's critical path) as two big strided
            # DMAs, then small token constants, all on the sync queue; the big
            # channel weights go on the scalar-engine HWDGE queue so they
            # don't block the token stage.
            # Per-batch x tiles: separate tiles so batch 0's consumers only
            # wait on batch 0's DMA. Startup-critical loads go first on sync;
            # big channel weights on the scalar queue.
            # Batches 0-3 on the sync queue, batches 4-7 on the scalar queue
            # AHEAD of the big channel weights: per-queue order is serial, so
            # the x tiles win HBM bandwidth at startup instead of striping
            # against 4.7MB of cwT/w4sT (which aren't needed until ~113us).
            xbs = []
            nc.sync.dma_start(twT[:], d["twT"])
            for b in range(BL):
                xb = cp.tile([128, 2, C], F16, name=f"xb{b}")
                q = nc.sync if b < 4 else nc.scalar
                q.dma_start(xb[:, 0, :], d["xa"][b])
                q.dma_start(xb[0:68, 1, :], d["xb"][b, 0:68, :])
                xbs.append(xb)
                if b == 0:
                    nc.sync.dma_start(w2sT[:], d["w2sT"])
                    nc.sync.dma_start(i196[:], d["i196"])
                    nc.sync.dma_start(b2r[0:1, :], d["b2r"])
                    nc.sync.dma_start(wnt[:], d["wnt"])
                    nc.sync.dma_start(tbc[:], d["tbc"])
            nc.sync.dma_start(wnc[:], d["wnc"])
            nc.sync.dma_start(cbc[:], d["cbc"])
            nc.scalar.dma_start(cwT[:], d["cwT"])
            nc.scalar.dma_start(w4sT[:], d["w4sT"])
            nc.scalar.dma_start(b4r[0:1, :], d["b4r"])
            nc.vector.memset(ones[:], 1.0)
            nc.vector.memset(x2T[:, :, ROWS:ROWSP], 0.0)

            # ================= Token stage =================
            with (
                tc.tile_pool(name="tok_sbuf", bufs=2) as tp,
                tc.tile_pool(name="tok_psum", bufs=1, space="PSUM") as pp,
            ):
                for b in range(BL):
                    r0 = b * P
                    xb = xbs[b]

                    # dot1 first: it only needs twT + x, so the PE can start
                    # before the norm chain is ready.
                    dot1s = []
                    for tcn in range(3):
                        ps_dot1 = pp.tile(
                            [128, C], F32, tag="ps_dot1", bufs=2, name="ps_dot1"
                        )
                        for kc, kn in ((0, 128), (1, 68)):
                            for no, nn_ in _n_slices(C):
                                nc.tensor.matmul(
                                    ps_dot1[:, no : no + nn_],
                                    twT[0:kn, kc, tcn * 128 : (tcn + 1) * 128],
                                    xb[0:kn, kc, no : no + nn_],
                                    start=(kc == 0),
                                    stop=(kc == 1),
                                )
                        dot1s.append(ps_dot1)

                    # x-norm broadcast tile: xnb[q, c] = sum_p x[p, c]^2
                    xsq = tp.tile([128, 2, C], F16, tag="xsq")
                    nc.vector.tensor_mul(xsq[:, 0, :], xb[:, 0, :], xb[:, 0, :])
                    nc.vector.tensor_mul(
                        xsq[0:68, 1, :], xb[0:68, 1, :], xb[0:68, 1, :]
                    )
                    ps_xnb = pp.tile([128, C], F32, tag="ps_xnb", bufs=1)
                    for no, nn_ in _n_slices(C):
                        nc.tensor.matmul(
                            ps_xnb[:, no : no + nn_],
                            ones[:, :],
                            xsq[:, 0, no : no + nn_],
                            start=True,
                            stop=False,
                        )
                        nc.tensor.matmul(
                            ps_xnb[:, no : no + nn_],
                            ones[0:68, :],
                            xsq[0:68, 1, no : no + nn_],
                            start=False,
                            stop=True,
                        )
                    xnb = tp.tile([128, C], F32, tag="xnb")
                    nc.scalar.copy(xnb[:], ps_xnb[:])

                    h1 = tp.tile([128, 3, C], F16, tag="h1")
                    for tcn in range(3):
                        ps_dot1 = dot1s[tcn]
                        den = tp.tile([128, C], F32, tag="den")
                        nc.vector.affine_then_add(
                            den[:], ps_dot1[:], xnb[:],
                            scale=-2.0, bias=wnt[:, tcn : tcn + 1],
                        )
                        rec = tp.tile([128, C], F32, tag="rec")
                        nc.vector.reciprocal_approx_fast(rec[:], den[:])
                        sq = tp.tile([128, C], F32, tag="sq")
                        nc.scalar.activation(
                            sq[:], ps_dot1[:], AF.Square, bias=tbc[:, tcn : tcn + 1]
                        )
                        nc.gpsimd.tensor_mul(h1[:, tcn, :], sq[:], rec[:])

                    # token linear + shortcut + bias -> x2T columns for batch b
                    for mc in range(6):
                        ps_x2 = pp.tile([128, P], F32, tag="ps_x2", bufs=2)
                        for kc in range(3):
                            nc.tensor.matmul(
                                ps_x2[:],
                                h1[:, kc, mc * 128 : (mc + 1) * 128],
                                w2sT[:, kc, :],
                                start=(kc == 0),
                                stop=False,
                            )
                        for kc, kn in ((0, 128), (1, 68)):
                            nc.tensor.matmul(
                                ps_x2[:],
                                xb[0:kn, kc, mc * 128 : (mc + 1) * 128],
                                i196[0:kn, kc, :],
                                start=False,
                                stop=False,
                            )
                        nc.tensor.matmul(
                            ps_x2[:],
                            ones[0:1, :],
                            b2r[0:1, :],
                            start=False,
                            stop=True,
                        )
                        nc.scalar.copy(x2T[:, mc, r0 : r0 + P], ps_x2[:])

            # ================= Channel-stage row norms =================
            with (
                tc.tile_pool(name="xn_sbuf", bufs=1) as xp,
                tc.tile_pool(name="xn_psum", bufs=1, space="PSUM") as xpp,
            ):
                ps_xn2 = xpp.tile([128, ROWS], F32)
                for kc in range(6):
                    x2sq = xp.tile([128, ROWS], F16, tag="x2sq", bufs=2)
                    nc.vector.tensor_mul(x2sq[:], x2T[:, kc, 0:ROWS], x2T[:, kc, 0:ROWS])
                    for no, nn_ in _n_slices(ROWS):
                        nc.tensor.matmul(
                            ps_xn2[:, no : no + nn_],
                            ones[:, :],
                            x2sq[:, no : no + nn_],
                            start=(kc == 0),
                            stop=(kc == 5),
                        )
                nc.scalar.copy(xn2b[:], ps_xn2[:])

            # ================= Channel stage =================
            with (
                tc.tile_pool(name="ch_sbuf", bufs=2) as chp,
                tc.tile_pool(name="ch_psum", bufs=1, space="PSUM") as cpp,
            ):
                for r0 in range(0, ROWS, RB):
                    rn = min(RB, ROWS - r0)
                    nsub = _ceil_div(rn, 128)
                    po = [
                        cpp.tile([128, C], F32, tag=f"po{s}", bufs=1, name=f"po{s}")
                        for s in range(nsub)
                    ]
                    for mc in range(24):
                        ps_d2 = cpp.tile([128, RB], F32, tag="ps_d2", bufs=4)
                        for kc in range(6):
                            nc.tensor.matmul(
                                ps_d2[:, 0:rn],
                                cwT[:, kc, mc * 128 : (mc + 1) * 128],
                                x2T[:, kc, r0 : r0 + rn],
                                start=(kc == 0),
                                stop=(kc == 5),
                            )
                        den2 = chp.tile([128, RB], F32, tag="den2", bufs=4)
                        nc.vector.affine_then_add(
                            den2[:, 0:rn], ps_d2[:, 0:rn], xn2b[:, r0 : r0 + rn],
                            scale=-2.0, bias=wnc[:, mc : mc + 1],
                        )
                        rec2 = chp.tile([128, RB], F32, tag="rec2", bufs=4)
                        nc.vector.reciprocal_approx_fast(rec2[:, 0:rn], den2[:, 0:rn])
                        sq2 = chp.tile([128, RB], F32, tag="sq2", bufs=4)
                        nc.scalar.activation(
                            sq2[:, 0:rn], ps_d2[:, 0:rn], AF.Square,
                            bias=cbc[:, mc : mc + 1],
                        )
                        h2 = chp.tile([128, RB], F16, tag="h2", bufs=4)
                        # Alternate the multiply between GPSIMD and DVE so the
                        # last link of the yat chain isn't serialized on one
                        # engine's FIFO.
                        mul_eng = nc.gpsimd if mc % 3 else nc.vector
                        mul_eng.tensor_mul(h2[:, 0:rn], sq2[:, 0:rn], rec2[:, 0:rn])

                        for s in range(nsub):
                            sn = min(128, rn - s * 128)
                            for no, nn_ in _n_slices(C):
                                nc.tensor.matmul(
                                    po[s][0:sn, no : no + nn_],
                                    h2[:, s * 128 : s * 128 + sn],
                                    w4sT[:, mc, no : no + nn_],
                                    start=(mc == 0),
                                    stop=False,
                                )
                    # bias b4 row, then shortcut x2 added via DVE from a
                    # DMA-transposed copy of x2T (cheaper than routing the
                    # identity through the PE).
                    for s in range(nsub):
                        sn = min(128, rn - s * 128)
                        rs = r0 + s * 128
                        for no, nn_ in _n_slices(C):
                            nc.tensor.matmul(
                                po[s][0:sn, no : no + nn_],
                                ones[0:1, 0:sn],
                                b4r[0:1, no : no + nn_],
                                start=False,
                                stop=True,
                            )
                        x2row = chp.tile([128, 6, 128], F16, tag="x2row", bufs=3)
                        for kc in range(6):
                            # Always a full 128-col source block (x2T free dim
                            # is padded); extra rows of x2row are unused.
                            # All on sync: the transpose issue occupies the
                            # host engine, and sync is otherwise idle (the
                            # scalar engine runs the critical Square ops).
                            nc.sync.dma_start_transpose(
                                x2row[:, kc, :], x2T[:, kc, rs : rs + 128]
                            )
                        osb = chp.tile([128, C], F16, tag="osb", bufs=3)
                        nc.vector.tensor_add(
                            osb[0:sn, :],
                            po[s][0:sn, :],
                            x2row[0:sn, :, :].rearrange("p a b -> p (a b)"),
                        )
                        nc.sync.dma_start(out_dram[rs : rs + sn, :], osb[0:sn, :])

    nc.compile()
    return nc


def _pack_kpn(w, n_chunks):
    """(K, N) fp32 -> (128, n_chunks, N) fp16 with zero padding of K."""
    k, n = w.shape
    out = np.zeros((n_chunks * 128, n), np.float16)
    out[:k] = w.astype(np.float16)
    return np.ascontiguousarray(
        out.reshape(n_chunks, 128, n).transpose(1, 0, 2)
    )


def _pack_col(v, n_chunks):
    """(K,) fp32 -> (128, n_chunks) fp32 column chunks."""
    out = np.zeros((n_chunks * 128,), np.float32)
    out[: v.shape[0]] = v.astype(np.float32)
    return np.ascontiguousarray(out.reshape(n_chunks, 128).T)


_PROGRAM = None


def _get_program():
    global _PROGRAM
    if _PROGRAM is None:
        _PROGRAM = build_program()
    return _PROGRAM


def kernel(x, tw, tb, t_alpha, w2, b2, cw, cb, c_alpha, w4, b4, _trace=False):
    x = np.asarray(x, np.float32)
    tw = np.asarray(tw, np.float32)
    tb = np.asarray(tb, np.float32)
    w2 = np.asarray(w2, np.float32)
    b2 = np.asarray(b2, np.float32)
    cw = np.asarray(cw, np.float32)
    cb = np.asarray(cb, np.float32)
    w4 = np.asarray(w4, np.float32)
    b4 = np.asarray(b4, np.float32)

    # YAT output scales (exactly as the reference computes them), folded into
    # the following linear layers' weights and biases' stays separate.
    scale_t = np.float32(np.sqrt(np.float32(T / np.log(T + 1.0)))) ** np.asarray(
        t_alpha, np.float32
    )[0]
    scale_c = np.float32(np.sqrt(np.float32(M3 / np.log(M3 + 1.0)))) ** np.asarray(
        c_alpha, np.float32
    )[0]
    w2s = (w2 * scale_t).astype(np.float32)   # (P, T)
    w4s = (w4 * scale_c).astype(np.float32)   # (C, M3)

    shared = {
        "twT": _pack_kpn(tw.T, 2),                       # (196,384) -> (128,2,384)
        "w2sT": _pack_kpn(w2s.T, 3),                     # (384,196) -> (128,3,196)
        "i196": _pack_kpn(np.eye(P, dtype=np.float32), 2),
        "b2r": b2.astype(np.float16).reshape(1, P),
        "cwT": _pack_kpn(cw.T, 6),                       # (768,3072)
        "w4sT": _pack_kpn(w4s.T, 24),                    # (3072,768)
        "b4r": b4.astype(np.float16).reshape(1, C),
        "wnt": _pack_col((tw.astype(np.float32) ** 2).sum(1) + EPS, 3),
        "tbc": _pack_col(tb, 3),
        "wnc": _pack_col((cw.astype(np.float32) ** 2).sum(1) + EPS, 24),
        "cbc": _pack_col(cb, 24),
    }
    x16 = x.astype(np.float16).reshape(NCORES, BL, P, C)
    xa = np.ascontiguousarray(x16[:, :, 0:128, :])
    xbp = np.zeros((NCORES, BL, 128, C), np.float16)
    xbp[:, :, 0:68] = x16[:, :, 128:P, :]
    in_maps = [dict(shared, xa=xa[c], xb=xbp[c]) for c in range(NCORES)]

    nc = _get_program()
    kwargs = {}
    if _trace:
        import shutil

        shutil.rmtree("/tmp/bass_ntff", ignore_errors=True)
        import os

        os.makedirs("/tmp/bass_ntff", exist_ok=True)
        kwargs["tmpdir"] = "/tmp/bass_ntff"
    res = bass_utils.run_bass_kernel_spmd(
        nc, in_maps, core_ids=list(range(NCORES)), trace=_trace, **kwargs
    )
    out = np.concatenate([res.results[c]["out"] for c in range(NCORES)], axis=0)
    out = out.reshape(B, P, C).astype(np.float32)
    if _trace:
        kernel.last_results = res
    return out



# revision 4
# speedup vs baseline: 1.3657x; 1.3657x over previous
"""Trainium2 Bass kernel for the YAT MixerBlock (nn_MixerBlock_12524124635797).

Data-parallel over batch (64 -> 8 per core); each core runs the full block
for its 8 batch elements (1568 rows of 768).

Key structure (vs naive):
  - Channel-mix GEMMs (dot2 = cw@x2, linear2 = h2@w4) run in fp8e4 with
    DoubleRow perf mode (2 k-chunks per instruction) on 512-row blocks.
  - The YAT chain is 2 fused custom DVE ops per tile:
      op1 YAT_DEN_RECIP_PS: rec = recip1(xn - 2*dot + wn)   (8-stage, reads PSUM)
      op2 YAT_NUM_SCALE:    h   = (dot - (-b))^2 * rec * s  (4-stage, fp8 out)
    Numerator for the token stage uses ACT Square(bias) + DVE mul instead
    (engine balance).
  - Channel-mix output is computed TRANSPOSED ([c-part, row-free]) by making
    w4 the stationary operand, so the residual add is a scaled-identity PE
    accumulation against x2T and the b4 bias + descale ride the final ACT
    copy. Output is DMA'd transposed and fixed up on host.
  - Token x-norms (pure function of the input x) are computed on host and
    DMA'd in broadcast form.
  - Token linear folds bias b2 via an extra ones-row of x against a b2 row
    appended to the identity (shortcut) matrix.
"""

import numpy as np
import ml_dtypes

import concourse.bass as bass
import concourse.bacc as bacc
import concourse.mybir as mybir
from concourse import bass_utils
from concourse import tile

F8 = mybir.dt.float8e4
F16 = mybir.dt.float16
BF16 = mybir.dt.bfloat16
F32 = mybir.dt.float32
AF = mybir.ActivationFunctionType
DR = mybir.MatmulPerfMode.DoubleRow
NPF8 = ml_dtypes.float8_e4m3
NPBF16 = ml_dtypes.bfloat16

EPS = 0.1
B, P, C, T, M3 = 64, 196, 768, 384, 3072
NCORES = 8
BL = B // NCORES          # 8 batches per core
ROWS = BL * P             # 1568 rows per core
ROWSP = 1664              # padded (mult of 128; keeps fp8 k-pair step %16==0)
RB = 512                  # channel row-block
BLOCKS = [(0, 512), (512, 512), (1024, 512), (1536, 32)]

S_W = 64.0                # cw fp8 scale
S_X = 16.0                # x2 fp8 scale
S_4 = 32.0                # w4s fp8 scale
ALPHA = 256.0             # h2 fp8 scale
SWX = S_W * S_X           # 1024
AS4 = ALPHA * S_4         # 8192 (identity-shortcut scale)

RECIP_C0 = -0.23549792
RECIP_C1 = 2.0017324

# ---------------- custom DVE ops ----------------

from concourse import dve_ops as DOPS
from concourse.dve_spec import Spec, Src0, Src1, C0, C1, C2, Bin, AluOp, lower, sq
from concourse.dve_spec import _has_src1 as _spec_has_src1
from concourse.dve_uop import DveOpSpec


def _register_dve_op(name, spec, subdim=False):
    for op in DOPS.OPS:
        if op.name == name:
            return op
    row = max(DOPS._SUB_OPCODE_FOR_NAME.values()) + 1
    assert row < 0x20, "no free custom-DVE opcode rows"
    op = DOPS.DveOp(name=name, spec=spec, subdim=subdim, uops_sha={})
    for ver in ("v3", "v4"):
        try:
            lowered = lower(spec, ver=ver)
            ospec = DveOpSpec(
                name=name, opcode=row, uops=lowered, rd1_en=_spec_has_src1(spec)
            )
            op.uops_sha[ver] = ospec.sha(ver)
        except Exception:
            pass
    DOPS.OPS.append(op)
    DOPS._SUB_OPCODE_FOR_NAME[name] = row
    DOPS.CUSTOM_DVE_SPECS[name] = spec
    return op


def _ref_den_recip(in0, in1, s0, s1, imm2):
    den = ((in1.astype(np.float32) - in0) - in0 + s0).astype(np.float32)
    nx = (~den.view(np.int32)).view(np.float32)
    y0 = nx * np.float32(s1)
    return y0 * (np.float32(imm2) - den * y0)


def _ref_num_scale(in0, in1, s0, s1, imm2):
    a = in0.astype(np.float32) - s0
    return (a * a) * in1.astype(np.float32) * np.float32(s1)


_den = ((Src1 - Src0) - Src0) + C0
_nx = Bin(AluOp.BITWISE_NOT, _den, _den)
_y0 = _nx * C1

OP_DEN_RECIP = _register_dve_op(
    "YAT_DEN_RECIP_PS",
    Spec(body=_y0 * (C2 - _den * _y0), reference=_ref_den_recip),
)
OP_NUM_SCALE = _register_dve_op(
    "YAT_NUM_SCALE",
    Spec(body=sq(Src0 - C0) * Src1 * C1, reference=_ref_num_scale),
)


def _n_slices(n, step=512):
    out = []
    o = 0
    while o < n:
        out.append((o, min(step, n - o)))
        o += step
    return out


def build_program():
    nc = bacc.Bacc(
        "TRN2",
        target_bir_lowering=False,
        debug=False,
        enable_asserts=False,
        num_devices=NCORES,
    )

    d = {}
    d["xball"] = nc.dram_tensor("xball", [BL, 128, 2, C], F16, kind="ExternalInput").ap()
    d["xn1b"] = nc.dram_tensor("xn1b", [BL, 128, C], BF16, kind="ExternalInput").ap()
    d["twT"] = nc.dram_tensor("twT", [128, 2, T], F16, kind="ExternalInput").ap()
    d["w2sT"] = nc.dram_tensor("w2sT", [128, 3, P], BF16, kind="ExternalInput").ap()
    d["i196b"] = nc.dram_tensor("i196b", [128, 2, P], F16, kind="ExternalInput").ap()
    d["cwT8"] = nc.dram_tensor("cwT8", [128, 6, M3], F8, kind="ExternalInput").ap()
    d["w4sT8"] = nc.dram_tensor("w4sT8", [128, 24, C], F8, kind="ExternalInput").ap()
    d["ones8c"] = nc.dram_tensor("ones8c", [128, 6, 128], F8, kind="ExternalInput").ap()
    d["ident"] = nc.dram_tensor("ident", [128, 128], F16, kind="ExternalInput").ap()
    d["wnt"] = nc.dram_tensor("wnt", [128, 3], F32, kind="ExternalInput").ap()
    d["tbc"] = nc.dram_tensor("tbc", [128, 3], F32, kind="ExternalInput").ap()
    d["wncs"] = nc.dram_tensor("wncs", [128, 24], F32, kind="ExternalInput").ap()
    d["cbcs"] = nc.dram_tensor("cbcs", [128, 24], F32, kind="ExternalInput").ap()
    d["b4c"] = nc.dram_tensor("b4c", [128, 6], F32, kind="ExternalInput").ap()
    out_dram = nc.dram_tensor("outT", [C, ROWS], F16, kind="ExternalOutput").ap()

    with tile.TileContext(nc) as tc:
        with tc.tile_pool(name="consts", bufs=1) as cp:
            twT = cp.tile([128, 2, T], F16)
            w2sT = cp.tile([128, 3, P], BF16)
            i196b = cp.tile([128, 2, P], F16)
            cwT8 = cp.tile([128, 6, M3], F8)
            w4sT8 = cp.tile([128, 24, C], F8)
            ones8c = cp.tile([128, 6, 128], F8)
            ident = cp.tile([128, 128], F16)
            wnt = cp.tile([128, 3], F32)
            tbc = cp.tile([128, 3], F32)
            wncs = cp.tile([128, 24], F32)
            cbcs = cp.tile([128, 24], F32)
            b4c = cp.tile([128, 6], F32)
            x2T16 = cp.tile([128, 6, ROWSP], F16)
            x2T8 = cp.tile([128, 6, ROWSP], F8)
            xn2b = cp.tile([128, ROWS], BF16)

            xbs = []
            xn1s = []
            for b in range(BL):
                xbs.append(cp.tile([128, 2, C], F16, name=f"xb{b}"))
                xn1s.append(cp.tile([128, C], BF16, name=f"xn1_{b}"))

            # --- input DMAs ---
            # sync: batch 0/1 + token-critical consts; vector: batch 2/3;
            # gpsimd (no compute role): batches 4-7 + channel weights.
            nc.sync.dma_start(twT[:], d["twT"])
            nc.sync.dma_start(xbs[0][:], d["xball"][0])
            nc.sync.dma_start(xn1s[0][:], d["xn1b"][0])
            nc.sync.dma_start(wnt[:], d["wnt"])
            nc.sync.dma_start(tbc[:], d["tbc"])
            nc.sync.dma_start(w2sT[:], d["w2sT"])
            nc.sync.dma_start(i196b[:], d["i196b"])
            nc.sync.dma_start(xbs[1][:], d["xball"][1])
            nc.sync.dma_start(xn1s[1][:], d["xn1b"][1])
            nc.sync.dma_start(xbs[2][:], d["xball"][2])
            nc.sync.dma_start(xn1s[2][:], d["xn1b"][2])
            for b in (3, 4, 5):
                nc.gpsimd.dma_start(xbs[b][:], d["xball"][b])
                nc.gpsimd.dma_start(xn1s[b][:], d["xn1b"][b])
            nc.gpsimd.dma_start(ones8c[:], d["ones8c"])
            for b in (6, 7):
                nc.gpsimd.dma_start(xbs[b][:], d["xball"][b])
                nc.gpsimd.dma_start(xn1s[b][:], d["xn1b"][b])
            nc.gpsimd.dma_start(ident[:], d["ident"])
            nc.gpsimd.dma_start(wncs[:], d["wncs"])
            nc.gpsimd.dma_start(cbcs[:], d["cbcs"])
            nc.gpsimd.dma_start(b4c[:], d["b4c"])
            nc.gpsimd.dma_start(cwT8[:], d["cwT8"])
            nc.gpsimd.dma_start(w4sT8[:], d["w4sT8"])

            # ================= Token stage =================
            with (
                tc.tile_pool(name="tok_sbuf", bufs=2) as tp,
                tc.tile_pool(name="tok_psum", bufs=1, space="PSUM") as pp,
            ):
                for b in range(BL):
                    r0 = b * P
                    xb = xbs[b]

                    dot1s = []
                    for tcn in range(3):
                        ps_dot1 = pp.tile([128, C], F32, tag="ps_dot1", bufs=2)
                        for kc, kn in ((0, 128), (1, 68)):
                            for no, nn_ in _n_slices(C):
                                nc.tensor.matmul(
                                    ps_dot1[:, no : no + nn_],
                                    twT[0:kn, kc, tcn * 128 : (tcn + 1) * 128],
                                    xb[0:kn, kc, no : no + nn_],
                                    start=(kc == 0),
                                    stop=(kc == 1),
                                )
                        dot1s.append(ps_dot1)

                    h1 = tp.tile([128, 3, C], BF16, tag="h1")
                    for tcn in range(3):
                        ps_dot1 = dot1s[tcn]
                        rec1 = tp.tile([128, C], BF16, tag="rec1", bufs=3)
                        nc.vector._custom_dve(
                            OP_DEN_RECIP,
                            out=rec1[:],
                            in0=ps_dot1[:],
                            in1=xn1s[b][:],
                            s0=wnt[:, tcn : tcn + 1],
                            s1=RECIP_C0,
                            imm2=RECIP_C1,
                        )
                        sq1 = tp.tile([128, C], BF16, tag="sq1", bufs=3)
                        nc.scalar.activation(
                            sq1[:], ps_dot1[:], AF.Square,
                            bias=tbc[:, tcn : tcn + 1],
                        )
                        nc.vector.tensor_mul(h1[:, tcn, :], sq1[:], rec1[:])

                    # token linear + shortcut + bias (bias rides the ones-row
                    # of xb chunk1 against the b2 row of i196b)
                    for mc in range(6):
                        ps_x2 = pp.tile([128, P], F32, tag="ps_x2", bufs=2)
                        for kc in range(3):
                            nc.tensor.matmul(
                                ps_x2[:],
                                h1[:, kc, mc * 128 : (mc + 1) * 128],
                                w2sT[:, kc, :],
                                start=(kc == 0),
                                stop=False,
                            )
                        for kc, kn in ((0, 128), (1, 69)):
                            nc.tensor.matmul(
                                ps_x2[:],
                                xb[0:kn, kc, mc * 128 : (mc + 1) * 128],
                                i196b[0:kn, kc, :],
                                start=False,
                                stop=(kc == 1),
                            )
                        nc.scalar.copy(x2T16[:, mc, r0 : r0 + P], ps_x2[:])

                    # x2 fp8 copy + row-norm accumulation for the channel stage
                    nc.vector.tensor_scalar_mul(
                        x2T8[:, :, r0 : r0 + P], x2T16[:, :, r0 : r0 + P], S_X
                    )
                    x2sq = tp.tile([128, 6, P], F8, tag="x2sq", bufs=2)
                    nc.vector.tensor_mul(
                        x2sq[:], x2T16[:, :, r0 : r0 + P], x2T16[:, :, r0 : r0 + P]
                    )
                    ps_xn2 = pp.tile([128, P], F32, tag="ps_xn2", bufs=2)
                    for j in range(3):
                        nc.tensor.matmul(
                            ps_xn2[:],
                            ones8c[:, 2 * j : 2 * j + 2, :],
                            x2sq[:, 2 * j : 2 * j + 2, :],
                            start=(j == 0),
                            stop=(j == 2),
                            perf_mode=DR,
                        )
                    # xn2b holds SWX * ||x2||^2 (op1's den is in SWX-scaled units)
                    nc.scalar.activation(
                        xn2b[:, r0 : r0 + P], ps_xn2[:], AF.Copy, scale=SWX
                    )

            # zero the padded x2T8 tail columns so tail-block fp8 GEMMs are clean
            nc.vector.memset(x2T8[:, :, ROWS:ROWSP], 0.0)

            # ================= Channel stage =================
            with (
                tc.tile_pool(name="ch_sbuf", bufs=2) as chp,
                tc.tile_pool(name="ch_psum", bufs=1, space="PSUM") as cpp,
            ):
                for r0, rn in BLOCKS:
                    po = [
                        cpp.tile([128, RB], F32, tag=f"po{s}", bufs=1, name=f"po{s}")
                        for s in range(6)
                    ]
                    h2p = None
                    for mc in range(24):
                        ps_d2 = cpp.tile([128, RB], F32, tag="ps_d2", bufs=2)
                        for j in range(3):
                            nc.tensor.matmul(
                                ps_d2[:, 0:rn],
                                cwT8[:, 2 * j : 2 * j + 2, mc * 128 : (mc + 1) * 128],
                                x2T8[:, 2 * j : 2 * j + 2, r0 : r0 + rn],
                                start=(j == 0),
                                stop=(j == 2),
                                perf_mode=DR,
                            )
                        if mc % 2 == 0:
                            h2p = chp.tile([128, 2, RB], F8, tag="h2p", bufs=2)
                        rec2 = chp.tile([128, RB], BF16, tag="rec2", bufs=4)
                        nc.vector._custom_dve(
                            OP_DEN_RECIP,
                            out=rec2[:, 0:rn],
                            in0=ps_d2[:, 0:rn],
                            in1=xn2b[:, r0 : r0 + rn],
                            s0=wncs[:, mc : mc + 1],
                            s1=RECIP_C0,
                            imm2=RECIP_C1,
                        )
                        nc.vector._custom_dve(
                            OP_NUM_SCALE,
                            out=h2p[:, mc % 2, 0:rn],
                            in0=ps_d2[:, 0:rn],
                            in1=rec2[:, 0:rn],
                            s0=cbcs[:, mc : mc + 1],
                            s1=ALPHA / SWX,
                            imm2=0.0,
                        )
                        if mc % 2 == 1:
                            for ch in range(6):
                                nc.tensor.matmul(
                                    po[ch][:, 0:rn],
                                    w4sT8[:, mc - 1 : mc + 1, ch * 128 : (ch + 1) * 128],
                                    h2p[:, 0:2, 0:rn],
                                    start=(mc == 1),
                                    stop=False,
                                    perf_mode=DR,
                                )
                    # residual (scaled identity) + drain with bias/descale
                    for ch in range(6):
                        nc.tensor.matmul(
                            po[ch][:, 0:rn],
                            ident[:],
                            x2T16[:, ch, r0 : r0 + rn],
                            start=False,
                            stop=True,
                        )
                        osb = chp.tile([128, RB], F16, tag="osb", bufs=3)
                        nc.scalar.activation(
                            osb[:, 0:rn], po[ch][:, 0:rn], AF.Identity,
                            scale=1.0 / AS4, bias=b4c[:, ch : ch + 1],
                        )
                        nc.sync.dma_start(
                            out_dram[ch * 128 : (ch + 1) * 128, r0 : r0 + rn],
                            osb[:, 0:rn],
                        )

    nc.compile()
    return nc


# ---------------- host packing ----------------


def _pack_kpn(w, n_chunks, np_dtype):
    k, n = w.shape
    out = np.zeros((n_chunks * 128, n), np.float32)
    out[:k] = w
    return np.ascontiguousarray(
        out.reshape(n_chunks, 128, n).transpose(1, 0, 2)
    ).astype(np_dtype)


def _pack_col(v, n_chunks):
    out = np.zeros((n_chunks * 128,), np.float32)
    out[: v.shape[0]] = v
    return np.ascontiguousarray(out.reshape(n_chunks, 128).T)


_PROGRAM = None


def _get_program():
    global _PROGRAM
    if _PROGRAM is None:
        _PROGRAM = build_program()
    return _PROGRAM


def kernel(x, tw, tb, t_alpha, w2, b2, cw, cb, c_alpha, w4, b4, _trace=False):
    x = np.asarray(x, np.float32)
    tw = np.asarray(tw, np.float32)
    tb = np.asarray(tb, np.float32)
    w2 = np.asarray(w2, np.float32)
    b2 = np.asarray(b2, np.float32)
    cw = np.asarray(cw, np.float32)
    cb = np.asarray(cb, np.float32)
    w4 = np.asarray(w4, np.float32)
    b4 = np.asarray(b4, np.float32)

    scale_t = np.float32(np.sqrt(np.float32(T / np.log(T + 1.0)))) ** np.asarray(
        t_alpha, np.float32
    )[0]
    scale_c = np.float32(np.sqrt(np.float32(M3 / np.log(M3 + 1.0)))) ** np.asarray(
        c_alpha, np.float32
    )[0]
    w2s = (w2 * scale_t).astype(np.float32)   # (P, T)
    w4s = (w4 * scale_c).astype(np.float32)   # (C, M3)

    # identity + b2 row for the token shortcut/bias matmul
    i196b = np.zeros((2 * 128, P), np.float32)
    i196b[:P] = np.eye(P, dtype=np.float32)
    i196b[128 + 68] = b2
    i196b = np.ascontiguousarray(
        i196b.reshape(2, 128, P).transpose(1, 0, 2)
    ).astype(np.float16)

    # ones lhsT for the channel row-norm reduction (full 768 k-rows)
    ones8c = np.ones((128, 6, 128), np.float32).astype(NPF8)

    ident = (np.eye(128, dtype=np.float32) * AS4).astype(np.float16)

    shared = {
        "twT": _pack_kpn(tw.T, 2, np.float16),
        "w2sT": _pack_kpn(w2s.T, 3, NPBF16),
        "i196b": i196b,
        "cwT8": _pack_kpn(np.clip(cw.T * S_W, -240, 240), 6, NPF8),
        "w4sT8": _pack_kpn(np.clip(w4s.T * S_4, -240, 240), 24, NPF8),
        "ones8c": ones8c,
        "ident": ident,
        "wnt": _pack_col((tw ** 2).sum(1) + EPS, 3),
        "tbc": _pack_col(tb, 3),
        # channel den/bias constants in SWX-scaled units
        "wncs": _pack_col(((cw ** 2).sum(1) + EPS) * SWX, 24),
        "cbcs": _pack_col(-cb * SWX, 24),
        "b4c": _pack_col(b4, 6),
    }

    # x tiles: [BL, 128, 2, C]; chunk1 row 68 = 1.0 (bias-trick ones row)
    x16 = x.astype(np.float16).reshape(NCORES, BL, P, C)
    xball = np.zeros((NCORES, BL, 128, 2, C), np.float16)
    xball[:, :, 0:128, 0, :] = x16[:, :, 0:128, :]
    xball[:, :, 0:68, 1, :] = x16[:, :, 128:P, :]
    xball[:, :, 68, 1, :] = 1.0

    # host-computed token x-norms (from the fp16 x actually used on device),
    # broadcast across partitions
    xf = x16.astype(np.float32)
    xn1 = (xf * xf).sum(axis=2)                       # (NCORES, BL, C)
    xn1b = np.broadcast_to(
        xn1[:, :, None, :], (NCORES, BL, 128, C)
    ).astype(NPBF16)

    in_maps = [
        dict(shared, xball=xball[c], xn1b=np.ascontiguousarray(xn1b[c]))
        for c in range(NCORES)
    ]

    nc = _get_program()
    kwargs = {}
    if _trace:
        import os
        import shutil

        shutil.rmtree("/tmp/bass_ntff", ignore_errors=True)
        os.makedirs("/tmp/bass_ntff", exist_ok=True)
        kwargs["tmpdir"] = "/tmp/bass_ntff"
    res = bass_utils.run_bass_kernel_spmd(
        nc, in_maps, core_ids=list(range(NCORES)), trace=_trace, **kwargs
    )
    out = np.stack(
        [res.results[c]["outT"] for c in range(NCORES)], axis=0
    )  # (NCORES, C, ROWS)
    out = out.astype(np.float32).transpose(0, 2, 1).reshape(B, P, C)
    if _trace:
        kernel.last_results = res
    return out


# revision 16
# speedup vs baseline: 1.7944x; 1.3140x over previous
"""Trainium2 Bass kernel for the YAT MixerBlock (nn_MixerBlock_12524124635797).

Data-parallel over batch (64 -> 8 per core); each core runs the full block
for its 8 batch elements (1568 rows of 768).

Key structure (vs naive):
  - Channel-mix GEMMs (dot2 = cw@x2, linear2 = h2@w4) run in fp8e4 with
    DoubleRow perf mode (2 k-chunks per instruction) on 512-row blocks.
  - The YAT chain is 2 fused custom DVE ops per tile:
      op1 YAT_DEN_RECIP_PS: rec = recip1(xn - 2*dot + wn)   (8-stage, reads PSUM)
      op2 YAT_NUM_SCALE:    h   = (dot - (-b))^2 * rec * s  (4-stage, fp8 out)
    Numerator for the token stage uses ACT Square(bias) + DVE mul instead
    (engine balance).
  - Channel-mix output is computed TRANSPOSED ([c-part, row-free]) by making
    w4 the stationary operand, so the residual add is a scaled-identity PE
    accumulation against x2T and the b4 bias + descale ride the final ACT
    copy. Output is DMA'd transposed and fixed up on host.
  - Token x-norms (pure function of the input x) are computed on host and
    DMA'd in broadcast form.
  - Token linear folds bias b2 via an extra ones-row of x against a b2 row
    appended to the identity (shortcut) matrix.
"""

import numpy as np
import ml_dtypes

import concourse.bass as bass
import concourse.bacc as bacc
import concourse.mybir as mybir
from concourse import bass_utils
from concourse import tile

F8 = mybir.dt.float8e4
F16 = mybir.dt.float16
BF16 = mybir.dt.bfloat16
F32 = mybir.dt.float32
AF = mybir.ActivationFunctionType
DR = mybir.MatmulPerfMode.DoubleRow
NPF8 = ml_dtypes.float8_e4m3
NPBF16 = ml_dtypes.bfloat16

EPS = 0.1
B, P, C, T, M3 = 64, 196, 768, 384, 3072
NCORES = 8
BL = B // NCORES          # 8 batches per core
ROWS = BL * P             # 1568 rows per core
ROWSP = 1664              # padded (mult of 128; keeps fp8 k-pair step %16==0)
RB = 392                  # channel row-block (4 uniform blocks, 1 PSUM bank each)
BLOCKS = [(0, 392), (392, 392), (784, 392), (1176, 392)]

S_W = 64.0                # cw fp8 scale
S_X = 16.0                # x2 fp8 scale
S_4 = 32.0                # w4s fp8 scale
ALPHA = 256.0             # h2 fp8 scale
SWX = S_W * S_X           # 1024
AS4 = ALPHA * S_4         # 8192 (identity-shortcut scale)

RECIP_C0 = -0.23549792
RECIP_C1 = 2.0017324

# ---------------- custom DVE ops ----------------

from concourse import dve_ops as DOPS
from concourse.dve_spec import Spec, Src0, Src1, C0, C1, C2, Bin, AluOp, lower, sq
from concourse.dve_spec import _has_src1 as _spec_has_src1
from concourse.dve_uop import DveOpSpec


def _register_dve_op(name, spec, subdim=False):
    for op in DOPS.OPS:
        if op.name == name:
            return op
    row = max(DOPS._SUB_OPCODE_FOR_NAME.values()) + 1
    assert row < 0x20, "no free custom-DVE opcode rows"
    op = DOPS.DveOp(name=name, spec=spec, subdim=subdim, uops_sha={})
    for ver in ("v3", "v4"):
        try:
            lowered = lower(spec, ver=ver)
            ospec = DveOpSpec(
                name=name, opcode=row, uops=lowered, rd1_en=_spec_has_src1(spec)
            )
            op.uops_sha[ver] = ospec.sha(ver)
        except Exception:
            pass
    DOPS.OPS.append(op)
    DOPS._SUB_OPCODE_FOR_NAME[name] = row
    DOPS.CUSTOM_DVE_SPECS[name] = spec
    return op


def _ref_den_recip(in0, in1, s0, s1, imm2):
    den = ((in1.astype(np.float32) - in0) - in0 + s0).astype(np.float32)
    nx = (~den.view(np.int32)).view(np.float32)
    y0 = nx * np.float32(s1)
    return y0 * (np.float32(imm2) - den * y0)


def _ref_num_scale(in0, in1, s0, s1, imm2):
    a = in0.astype(np.float32) - s0
    return (a * a) * in1.astype(np.float32) * np.float32(s1)


def _ref_den_recip_t(in0, in1, s0, s1, imm2):
    den = (in0.astype(np.float32) + in1.astype(np.float32) + s0).astype(np.float32)
    nx = (~den.view(np.int32)).view(np.float32)
    y0 = nx * np.float32(s1)
    return y0 * (np.float32(imm2) - den * y0)


_den = ((Src1 - Src0) - Src0) + C0
_nx = Bin(AluOp.BITWISE_NOT, _den, _den)
_y0 = _nx * C1

OP_DEN_RECIP = _register_dve_op(
    "YAT_DEN_RECIP_PS",
    Spec(body=_y0 * (C2 - _den * _y0), reference=_ref_den_recip),
)

_dent = (Src0 + Src1) + C0
_nxt = Bin(AluOp.BITWISE_NOT, _dent, _dent)
_y0t = _nxt * C1

OP_DEN_RECIP_T = _register_dve_op(
    "YAT_DEN_RECIP_T",
    Spec(body=_y0t * (C2 - _dent * _y0t), reference=_ref_den_recip_t),
)

OP_NUM_SCALE = _register_dve_op(
    "YAT_NUM_SCALE",
    Spec(body=sq(Src0 - C0) * Src1 * C1, reference=_ref_num_scale),
)


def _n_slices(n, step=512):
    out = []
    o = 0
    while o < n:
        out.append((o, min(step, n - o)))
        o += step
    return out


def build_program():
    nc = bacc.Bacc(
        "TRN2",
        target_bir_lowering=False,
        debug=False,
        enable_asserts=False,
        num_devices=NCORES,
    )

    d = {}
    d["xball"] = nc.dram_tensor("xball", [BL, 128, 2, C], F16, kind="ExternalInput").ap()
    d["xn1b"] = nc.dram_tensor("xn1b", [BL, 128, C], BF16, kind="ExternalInput").ap()
    d["twT"] = nc.dram_tensor("twT", [128, 2, T], F16, kind="ExternalInput").ap()
    d["w2sT"] = nc.dram_tensor("w2sT", [128, 3, P], BF16, kind="ExternalInput").ap()
    d["i196b"] = nc.dram_tensor("i196b", [128, 2, P], F16, kind="ExternalInput").ap()
    d["cwT8"] = nc.dram_tensor("cwT8", [128, 6, M3], F8, kind="ExternalInput").ap()
    d["w4sT8"] = nc.dram_tensor("w4sT8", [128, 24, C], F8, kind="ExternalInput").ap()
    d["ones8c"] = nc.dram_tensor("ones8c", [128, 6, 128], F8, kind="ExternalInput").ap()
    d["ident"] = nc.dram_tensor("ident", [128, 128], F16, kind="ExternalInput").ap()
    d["wnt"] = nc.dram_tensor("wnt", [128, 3], F32, kind="ExternalInput").ap()
    d["tbc"] = nc.dram_tensor("tbc", [128, 3], F32, kind="ExternalInput").ap()
    d["wncr"] = nc.dram_tensor("wncr", [128, 24], F32, kind="ExternalInput").ap()
    d["cb2"] = nc.dram_tensor("cb2", [128, 24], F32, kind="ExternalInput").ap()
    d["cbp16"] = nc.dram_tensor("cbp16", [128, 24], F32, kind="ExternalInput").ap()
    d["b4c"] = nc.dram_tensor("b4c", [128, 6], F32, kind="ExternalInput").ap()
    out_dram = nc.dram_tensor("outT", [C, ROWS], F16, kind="ExternalOutput").ap()

    with tile.TileContext(nc) as tc:
        with tc.tile_pool(name="consts", bufs=1) as cp:
            twT = cp.tile([128, 2, T], F16)
            w2sT = cp.tile([128, 3, P], BF16)
            i196b = cp.tile([128, 2, P], F16)
            cwT8 = cp.tile([128, 6, M3], F8)
            w4sT8 = cp.tile([128, 24, C], F8)
            ones8c = cp.tile([128, 6, 128], F8)
            ident = cp.tile([128, 128], F16)
            wnt = cp.tile([128, 3], F32)
            tbc = cp.tile([128, 3], F32)
            wncr = cp.tile([128, 24], F32)
            cb2 = cp.tile([128, 24], F32)
            cbp16 = cp.tile([128, 24], F32)
            b4c = cp.tile([128, 6], F32)
            x2T16 = cp.tile([128, 6, ROWSP], F16)
            x2T8 = cp.tile([128, 6, ROWSP], F8)
            xn2b = cp.tile([128, ROWS], BF16)

            xbs = []
            xn1s = []
            for b in range(BL):
                xbs.append(cp.tile([128, 2, C], F16, name=f"xb{b}"))
                xn1s.append(cp.tile([128, C], BF16, name=f"xn1_{b}"))

            # --- input DMAs ---
            # sync: batch 0/1 + token-critical consts; vector: batch 2/3;
            # gpsimd (no compute role): batches 4-7 + channel weights.
            nc.sync.dma_start(twT[:], d["twT"])
            nc.sync.dma_start(xbs[0][:], d["xball"][0])
            nc.sync.dma_start(xn1s[0][:], d["xn1b"][0])
            nc.sync.dma_start(wnt[:], d["wnt"])
            nc.sync.dma_start(tbc[:], d["tbc"])
            nc.sync.dma_start(w2sT[:], d["w2sT"])
            nc.sync.dma_start(i196b[:], d["i196b"])
            nc.sync.dma_start(xbs[1][:], d["xball"][1])
            nc.sync.dma_start(xn1s[1][:], d["xn1b"][1])
            nc.sync.dma_start(xbs[2][:], d["xball"][2])
            nc.sync.dma_start(xn1s[2][:], d["xn1b"][2])
            for b in (3, 4, 5):
                nc.gpsimd.dma_start(xbs[b][:], d["xball"][b])
                nc.gpsimd.dma_start(xn1s[b][:], d["xn1b"][b])
            nc.gpsimd.dma_start(ones8c[:], d["ones8c"])
            for b in (6, 7):
                nc.gpsimd.dma_start(xbs[b][:], d["xball"][b])
                nc.gpsimd.dma_start(xn1s[b][:], d["xn1b"][b])
            nc.gpsimd.dma_start(ident[:], d["ident"])
            nc.gpsimd.dma_start(wncr[:], d["wncr"])
            nc.gpsimd.dma_start(cb2[:], d["cb2"])
            nc.gpsimd.dma_start(cbp16[:], d["cbp16"])
            nc.gpsimd.dma_start(b4c[:], d["b4c"])
            nc.gpsimd.dma_start(cwT8[:], d["cwT8"])
            nc.gpsimd.dma_start(w4sT8[:], d["w4sT8"])

            # ================= Token stage =================
            with (
                tc.tile_pool(name="tok_sbuf", bufs=2) as tp,
                tc.tile_pool(name="tok_psum", bufs=1, space="PSUM") as pp,
            ):
                for b in range(BL):
                    r0 = b * P
                    xb = xbs[b]

                    dot1s = []
                    for tcn in range(3):
                        ps_dot1 = pp.tile([128, C], F32, tag="ps_dot1", bufs=2)
                        for kc, kn in ((0, 128), (1, 68)):
                            for no, nn_ in _n_slices(C):
                                nc.tensor.matmul(
                                    ps_dot1[:, no : no + nn_],
                                    twT[0:kn, kc, tcn * 128 : (tcn + 1) * 128],
                                    xb[0:kn, kc, no : no + nn_],
                                    start=(kc == 0),
                                    stop=(kc == 1),
                                )
                        dot1s.append(ps_dot1)

                    h1 = tp.tile([128, 3, C], BF16, tag="h1")
                    for tcn in range(3):
                        ps_dot1 = dot1s[tcn]
                        rec1 = tp.tile([128, C], BF16, tag="rec1", bufs=3)
                        nc.vector._custom_dve(
                            OP_DEN_RECIP,
                            out=rec1[:],
                            in0=ps_dot1[:],
                            in1=xn1s[b][:],
                            s0=wnt[:, tcn : tcn + 1],
                            s1=RECIP_C0,
                            imm2=RECIP_C1,
                        )
                        sq1 = tp.tile([128, C], BF16, tag="sq1", bufs=3)
                        nc.scalar.activation(
                            sq1[:], ps_dot1[:], AF.Square,
                            bias=tbc[:, tcn : tcn + 1],
                        )
                        nc.vector.tensor_mul(h1[:, tcn, :], sq1[:], rec1[:])

                    # token linear + shortcut + bias (bias rides the ones-row
                    # of xb chunk1 against the b2 row of i196b); mc pairs share
                    # a PSUM tile so the copy-out is one bigger ACT op
                    for mcp in range(3):
                        ps_x2 = pp.tile([128, 2, P], F32, tag="ps_x2", bufs=2)
                        for half in range(2):
                            mc = 2 * mcp + half
                            for kc in range(3):
                                nc.tensor.matmul(
                                    ps_x2[:, half, :],
                                    h1[:, kc, mc * 128 : (mc + 1) * 128],
                                    w2sT[:, kc, :],
                                    start=(kc == 0),
                                    stop=False,
                                )
                            for kc, kn in ((0, 128), (1, 69)):
                                nc.tensor.matmul(
                                    ps_x2[:, half, :],
                                    xb[0:kn, kc, mc * 128 : (mc + 1) * 128],
                                    i196b[0:kn, kc, :],
                                    start=False,
                                    stop=(kc == 1),
                                )
                        nc.scalar.copy(
                            x2T16[:, 2 * mcp : 2 * mcp + 2, r0 : r0 + P], ps_x2[:]
                        )

                    # x2 fp8 copy + row-norm accumulation for the channel stage
                    nc.vector.tensor_scalar_mul(
                        x2T8[:, :, r0 : r0 + P], x2T16[:, :, r0 : r0 + P], S_X
                    )
                    x2sq = tp.tile([128, 6, P], F8, tag="x2sq", bufs=2)
                    if b % 2 == 0:
                        nc.scalar.activation(
                            x2sq[:], x2T16[:, :, r0 : r0 + P], AF.Square
                        )
                    else:
                        nc.vector.tensor_mul(
                            x2sq[:], x2T16[:, :, r0 : r0 + P],
                            x2T16[:, :, r0 : r0 + P],
                        )
                    ps_xn2 = pp.tile([128, P], F32, tag="ps_xn2", bufs=2)
                    for j in range(3):
                        nc.tensor.matmul(
                            ps_xn2[:],
                            ones8c[:, 2 * j : 2 * j + 2, :],
                            x2sq[:, 2 * j : 2 * j + 2, :],
                            start=(j == 0),
                            stop=(j == 2),
                            perf_mode=DR,
                        )
                    nc.scalar.copy(xn2b[:, r0 : r0 + P], ps_xn2[:])

            # zero the padded x2T8 tail columns so tail-block fp8 GEMMs are clean
            nc.vector.memset(x2T8[:, :, ROWS:ROWSP], 0.0)

            # ================= Channel stage =================
            with (
                tc.tile_pool(name="ch_sbuf", bufs=2) as chp,
                tc.tile_pool(name="ch_psum", bufs=1, space="PSUM") as cpp,
            ):
                for r0, rn in BLOCKS:
                    po = [
                        cpp.tile([128, RB], F32, tag=f"po{s}", bufs=1, name=f"po{s}")
                        for s in range(6)
                    ]
                    h2p = None
                    for mc in range(24):
                        ps_d2 = cpp.tile([128, RB], F32, tag="ps_d2", bufs=2)
                        for j in range(3):
                            nc.tensor.matmul(
                                ps_d2[:, 0:rn],
                                cwT8[:, 2 * j : 2 * j + 2, mc * 128 : (mc + 1) * 128],
                                x2T8[:, 2 * j : 2 * j + 2, r0 : r0 + rn],
                                start=(j == 0),
                                stop=(j == 2),
                                perf_mode=DR,
                            )
                        if mc % 2 == 0:
                            h2p = chp.tile([128, 2, RB], F8, tag="h2p", bufs=2)
                        # t2 = -2*dot (true units); sole, fast PSUM reader so the
                        # next dot2 can reuse the bank immediately
                        t2 = chp.tile([128, RB], BF16, tag="t2", bufs=4)
                        nc.scalar.activation(
                            t2[:, 0:rn], ps_d2[:, 0:rn], AF.Copy, scale=-2.0 / SWX
                        )
                        rec2 = chp.tile([128, RB], BF16, tag="rec2", bufs=4)
                        nc.vector._custom_dve(
                            OP_DEN_RECIP_T,
                            out=rec2[:, 0:rn],
                            in0=t2[:, 0:rn],
                            in1=xn2b[:, r0 : r0 + rn],
                            s0=wncr[:, mc : mc + 1],
                            s1=RECIP_C0,
                            imm2=RECIP_C1,
                        )
                        if mc % 2 == 0:
                            # variant A: fused (t2-2cb)^2*rec*(alpha/4) on DVE
                            nc.vector._custom_dve(
                                OP_NUM_SCALE,
                                out=h2p[:, 0, 0:rn],
                                in0=t2[:, 0:rn],
                                in1=rec2[:, 0:rn],
                                s0=cb2[:, mc : mc + 1],
                                s1=ALPHA / 4.0,
                                imm2=0.0,
                            )
                        else:
                            # variant B: ACT square + GPSIMD mul (engine balance)
                            sqb = chp.tile([128, RB], BF16, tag="sqb", bufs=3)
                            nc.scalar.activation(
                                sqb[:, 0:rn], t2[:, 0:rn], AF.Square,
                                scale=-8.0, bias=cbp16[:, mc : mc + 1],
                            )
                            nc.gpsimd.tensor_mul(
                                h2p[:, 1, 0:rn], sqb[:, 0:rn], rec2[:, 0:rn]
                            )
                        if mc % 2 == 1:
                            for ch in range(6):
                                nc.tensor.matmul(
                                    po[ch][:, 0:rn],
                                    w4sT8[:, mc - 1 : mc + 1, ch * 128 : (ch + 1) * 128],
                                    h2p[:, 0:2, 0:rn],
                                    start=(mc == 1),
                                    stop=False,
                                    perf_mode=DR,
                                )
                    # residual (scaled identity) + drain with bias/descale
                    for ch in range(6):
                        nc.tensor.matmul(
                            po[ch][:, 0:rn],
                            ident[:],
                            x2T16[:, ch, r0 : r0 + rn],
                            start=False,
                            stop=True,
                        )
                        osb = chp.tile([128, RB], F16, tag="osb", bufs=3)
                        nc.scalar.activation(
                            osb[:, 0:rn], po[ch][:, 0:rn], AF.Identity,
                            scale=1.0 / AS4, bias=b4c[:, ch : ch + 1],
                        )
                        nc.sync.dma_start(
                            out_dram[ch * 128 : (ch + 1) * 128, r0 : r0 + rn],
                            osb[:, 0:rn],
                        )

    nc.compile()
    return nc


# ---------------- host packing ----------------


def _pack_kpn(w, n_chunks, np_dtype):
    k, n = w.shape
    out = np.zeros((n_chunks * 128, n), np.float32)
    out[:k] = w
    return np.ascontiguousarray(
        out.reshape(n_chunks, 128, n).transpose(1, 0, 2)
    ).astype(np_dtype)


def _pack_col(v, n_chunks):
    out = np.zeros((n_chunks * 128,), np.float32)
    out[: v.shape[0]] = v
    return np.ascontiguousarray(out.reshape(n_chunks, 128).T)


_PROGRAM = None


def _get_program():
    global _PROGRAM
    if _PROGRAM is None:
        _PROGRAM = build_program()
    return _PROGRAM


def kernel(x, tw, tb, t_alpha, w2, b2, cw, cb, c_alpha, w4, b4, _trace=False):
    x = np.asarray(x, np.float32)
    tw = np.asarray(tw, np.float32)
    tb = np.asarray(tb, np.float32)
    w2 = np.asarray(w2, np.float32)
    b2 = np.asarray(b2, np.float32)
    cw = np.asarray(cw, np.float32)
    cb = np.asarray(cb, np.float32)
    w4 = np.asarray(w4, np.float32)
    b4 = np.asarray(b4, np.float32)

    scale_t = np.float32(np.sqrt(np.float32(T / np.log(T + 1.0)))) ** np.asarray(
        t_alpha, np.float32
    )[0]
    scale_c = np.float32(np.sqrt(np.float32(M3 / np.log(M3 + 1.0)))) ** np.asarray(
        c_alpha, np.float32
    )[0]
    w2s = (w2 * scale_t).astype(np.float32)   # (P, T)
    w4s = (w4 * scale_c).astype(np.float32)   # (C, M3)

    # identity + b2 row for the token shortcut/bias matmul
    i196b = np.zeros((2 * 128, P), np.float32)
    i196b[:P] = np.eye(P, dtype=np.float32)
    i196b[128 + 68] = b2
    i196b = np.ascontiguousarray(
        i196b.reshape(2, 128, P).transpose(1, 0, 2)
    ).astype(np.float16)

    # ones lhsT for the channel row-norm reduction (full 768 k-rows)
    ones8c = np.ones((128, 6, 128), np.float32).astype(NPF8)

    ident = (np.eye(128, dtype=np.float32) * AS4).astype(np.float16)

    shared = {
        "twT": _pack_kpn(tw.T, 2, np.float16),
        "w2sT": _pack_kpn(w2s.T, 3, NPBF16),
        "i196b": i196b,
        "cwT8": _pack_kpn(np.clip(cw.T * S_W, -240, 240), 6, NPF8),
        "w4sT8": _pack_kpn(np.clip(w4s.T * S_4, -240, 240), 24, NPF8),
        "ones8c": ones8c,
        "ident": ident,
        "wnt": _pack_col((tw ** 2).sum(1) + EPS, 3),
        "tbc": _pack_col(tb, 3),
        "wncr": _pack_col((cw ** 2).sum(1) + EPS, 24),
        "cb2": _pack_col(2.0 * cb, 24),
        "cbp16": _pack_col(16.0 * cb, 24),
        "b4c": _pack_col(b4, 6),
    }

    # x tiles: [BL, 128, 2, C]; chunk1 row 68 = 1.0 (bias-trick ones row)
    x16 = x.astype(np.float16).reshape(NCORES, BL, P, C)
    xball = np.zeros((NCORES, BL, 128, 2, C), np.float16)
    xball[:, :, 0:128, 0, :] = x16[:, :, 0:128, :]
    xball[:, :, 0:68, 1, :] = x16[:, :, 128:P, :]
    xball[:, :, 68, 1, :] = 1.0

    # host-computed token x-norms (from the fp16 x actually used on device),
    # broadcast across partitions
    xf = x16.astype(np.float32)
    xn1 = (xf * xf).sum(axis=2)                       # (NCORES, BL, C)
    xn1b = np.broadcast_to(
        xn1[:, :, None, :], (NCORES, BL, 128, C)
    ).astype(NPBF16)

    in_maps = [
        dict(shared, xball=xball[c], xn1b=np.ascontiguousarray(xn1b[c]))
        for c in range(NCORES)
    ]

    nc = _get_program()
    kwargs = {}
    if _trace:
        import os
        import shutil

        shutil.rmtree("/tmp/bass_ntff", ignore_errors=True)
        os.makedirs("/tmp/bass_ntff", exist_ok=True)
        kwargs["tmpdir"] = "/tmp/bass_ntff"
    res = bass_utils.run_bass_kernel_spmd(
        nc, in_maps, core_ids=list(range(NCORES)), trace=_trace, **kwargs
    )
    out = np.stack(
        [res.results[c]["outT"] for c in range(NCORES)], axis=0
    )  # (NCORES, C, ROWS)
    out = out.astype(np.float32).transpose(0, 2, 1).reshape(B, P, C)
    if _trace:
        kernel.last_results = res
    return out
